# revision 10
# baseline (speedup 1.0000x reference)
"""MultiHeadAttention Trainium2 kernel (8 NeuronCores, SPMD, no collectives).

Problem: B=2, S=2048, E=512, H=8, Dh=64.  reference returns (out, weights):
  out     [B, S, E]      = softmax(q k^T / sqrt(Dh)) v  projected by Wo
  weights [B, H, S, S]   = the softmax attention weights (f32, 134 MB -> the
                           memory-traffic bottleneck; target_regime=memory)

Sharding: query-chunk data parallel.  Core c handles batch b=c//4 and query
rows [qc*512, (qc+1)*512) with qc=c%4, for ALL heads.  Each core computes
k/v for its full batch (recompute instead of collectives) and owns disjoint
slices of both outputs, so there is no cross-core communication.

Per-core plan (matmuls in bf16: full PE rate, FWL weight loads, keeps the
HAM clock-gate warm; accumulation is always f32 in PSUM):
  xbT = transpose(x[b]) via PE         [E,S] layout, E on partitions
  qT  = Wq^T xqT + bq                  [E,SQ]    (stored bf16)
  kT  = Wk^T xbT + bk                  [E,S]     (stored bf16)
  v   = x[b] Wv + bv                   [S,E]     (stored bf16)
  per head h:
    orientation 1 (weights output): scores[q,k] -> one FD=2048 exp on ACT
      (PSUM -> SBUF f32, accumulate register -> row sums) -> x(1/sum)
      (DVE per-partition scalar) -> DMA to w_out.  Max-subtraction is
      skipped (scores ~ N(0,1), exp range is tiny).
    1/sums are PE-transposed and DMA-bounced through DRAM into a
      partition-replicated tile (DVE cannot broadcast across partitions)
      to normalize the attention path.
    orientation 2 (attn): scoresT[k,q] -> exp -> pT (bf16); attn^T
      accumulated on PE with lhsT = v slice, dst partitions 0:64.
  out = sum_h concat_h^T Wo_h + bo -> DMA o_out   (K=64 chunks per head).
"""

import numpy as np

P = 128
S = 2048
E = 512
H = 8
DH = 64
SQ = 512          # queries per core
B = 2
NCORES = 8
SCALE = 0.125     # 1/sqrt(DH)

_CACHE = {}


def _build():
    import concourse.mybir as mybir
    import concourse.tile as tile
    from concourse import bacc
    from concourse.masks import make_identity
    from contextlib import ExitStack

    F32 = mybir.dt.float32
    BF16 = mybir.dt.bfloat16
    EXP = mybir.ActivationFunctionType.Exp
    MUL = mybir.AluOpType.mult
    ADD = mybir.AluOpType.add

    nc = bacc.Bacc(
        "TRN2", target_bir_lowering=False, debug=False,
        enable_asserts=False, num_devices=NCORES,
    )

    xb = nc.dram_tensor("xb", [S, E], F32, kind="ExternalInput").ap()
    xq = nc.dram_tensor("xq", [SQ, E], F32, kind="ExternalInput").ap()
    wq = nc.dram_tensor("wq", [E, E], F32, kind="ExternalInput").ap()
    wk = nc.dram_tensor("wk", [E, E], F32, kind="ExternalInput").ap()
    wv = nc.dram_tensor("wv", [E, E], F32, kind="ExternalInput").ap()
    wo = nc.dram_tensor("wo", [E, E], F32, kind="ExternalInput").ap()
    bq = nc.dram_tensor("bq", [E], F32, kind="ExternalInput").ap()
    bk = nc.dram_tensor("bk", [E], F32, kind="ExternalInput").ap()
    bv = nc.dram_tensor("bv", [E], F32, kind="ExternalInput").ap()
    bo = nc.dram_tensor("bo", [E], F32, kind="ExternalInput").ap()
    w_out = nc.dram_tensor("w_out", [H, SQ, S], F32, kind="ExternalOutput").ap()
    o_out = nc.dram_tensor("o_out", [SQ, E], F32, kind="ExternalOutput").ap()

    with tile.TileContext(nc) as tc, ExitStack() as ctx:
        consts = ctx.enter_context(tc.tile_pool(name="consts", bufs=1))
        ld = ctx.enter_context(tc.tile_pool(name="ld", bufs=4))
        wts = ctx.enter_context(tc.tile_pool(name="wts", bufs=1))
        big = ctx.enter_context(tc.tile_pool(name="big", bufs=1))
        slabs = ctx.enter_context(tc.tile_pool(name="slabs", bufs=5))
        ptp = ctx.enter_context(tc.tile_pool(name="ptp", bufs=4))
        stats = ctx.enter_context(tc.tile_pool(name="stats", bufs=1))
        invt = ctx.enter_context(tc.tile_pool(name="invt", bufs=2))
        invbc = ctx.enter_context(tc.tile_pool(name="invbc", bufs=3))
        osbp = ctx.enter_context(tc.tile_pool(name="osbp", bufs=2))
        dramp = ctx.enter_context(tc.tile_pool(name="dramp", bufs=2, space="DRAM"))
        # PSUM: ps_sm 2 banks (phase 0/1, inv transpose, attn accumulator,
        # out projection) + s1 4 banks + s2 2 banks = 8 banks total.
        ps_sm = ctx.enter_context(tc.tile_pool(name="ps_sm", bufs=1, space="PSUM"))
        ps_warm = ctx.enter_context(tc.tile_pool(name="ps_warm", bufs=1, space="PSUM"))
        ps_s1 = ctx.enter_context(tc.tile_pool(name="ps_s1", bufs=1, space="PSUM"))
        ps_s2 = ctx.enter_context(tc.tile_pool(name="ps_s2", bufs=1, space="PSUM"))

        # ---- constants -------------------------------------------------
        ident = consts.tile([P, P], BF16)
        make_identity(nc, ident)
        identf = consts.tile([P, P], F32)
        make_identity(nc, identf)
        bqt = consts.tile([P, 4], F32)
        nc.sync.dma_start(bqt, bq.rearrange("(o i) -> i o", i=P))
        bkt = consts.tile([P, 4], F32)
        nc.sync.dma_start(bkt, bk.rearrange("(o i) -> i o", i=P))
        bv_bc = consts.tile([P, E], F32)
        nc.sync.dma_start(bv_bc, bv[None, :].to_broadcast((P, E)))
        bo_bc = consts.tile([P, E], F32)
        nc.sync.dma_start(bo_bc, bo[None, :].to_broadcast((P, E)))

        # ---- weights (cast to bf16 during DMA; SWDGE) ------------------
        wv_t = wts.tile([P, 4, E], BF16, tag="wv")
        nc.gpsimd.dma_start(wv_t, wv.rearrange("(ko ki) n -> ki ko n", ki=P))
        # Wo head-major at partitions 0:64 (out-projection contracts per-head
        # K=64 chunks; matmul dst must start at partition 0).
        wo_t = wts.tile([64, H, E], BF16, tag="wo")
        nc.gpsimd.dma_start(wo_t, wo.rearrange("(h ki) n -> ki h n", ki=64))

        # ---- transpose x (bf16 via DMA cast, PE transpose) -------------
        xbT = big.tile([P, 4, S], BF16, tag="xbT")
        xb_r = xb.rearrange("(sc j si) e -> si sc j e", si=P, j=2)
        for sc in range(8):
            ch = ld.tile([P, 2, E], BF16, tag="ldc", name="xbc")
            nc.gpsimd.dma_start(ch, xb_r[:, sc])
            for j in range(2):
                so = sc * 2 + j
                tp = ps_sm.tile([P, 4, P], BF16, tag="ps", name="tp")
                for eo in range(4):
                    nc.tensor.transpose(tp[:, eo, :], ch[:, j, eo * P:(eo + 1) * P], ident)
                nc.vector.tensor_copy(xbT[:, :, so * P:(so + 1) * P], tp)

        xqT = big.tile([P, 4, SQ], BF16, tag="xqT")
        xq_r = xq.rearrange("(sc j si) e -> si sc j e", si=P, j=2)
        for sc in range(2):
            ch = ld.tile([P, 2, E], BF16, tag="ldc", name="xqc")
            nc.gpsimd.dma_start(ch, xq_r[:, sc])
            for j in range(2):
                so = sc * 2 + j
                tp = ps_sm.tile([P, 4, P], BF16, tag="ps", name="tq")
                for eo in range(4):
                    nc.tensor.transpose(tp[:, eo, :], ch[:, j, eo * P:(eo + 1) * P], ident)
                nc.vector.tensor_copy(xqT[:, :, so * P:(so + 1) * P], tp)

        wq_t = ld.tile([P, 4, E], BF16, tag="ldc", name="wq_t")
        nc.gpsimd.dma_start(wq_t, wq.rearrange("(ko ki) n -> ki ko n", ki=P))
        wk_t = ld.tile([P, 4, E], BF16, tag="ldc", name="wk_t")
        nc.gpsimd.dma_start(wk_t, wk.rearrange("(ko ki) n -> ki ko n", ki=P))

        # ---- projections (bf16 matmuls, f32 PSUM accumulation) ---------
        qT = big.tile([P, 4, SQ], BF16, tag="qT")
        for eo in range(4):
            pq = ps_sm.tile([P, E], F32, tag="ps", name="pq")
            for k in range(4):
                nc.tensor.matmul(pq, wq_t[:, k, eo * P:(eo + 1) * P], xqT[:, k, :],
                                 start=(k == 0), stop=(k == 3))
            nc.vector.tensor_scalar_add(qT[:, eo, :], pq, bqt[:, eo:eo + 1])

        kT = big.tile([P, 4, S], BF16, tag="kT")
        for eo in range(4):
            for sc in range(4):
                pk = ps_sm.tile([P, E], F32, tag="ps", name="pk")
                for k in range(4):
                    nc.tensor.matmul(pk, wk_t[:, k, eo * P:(eo + 1) * P],
                                     xbT[:, k, sc * E:(sc + 1) * E],
                                     start=(k == 0), stop=(k == 3))
                nc.vector.tensor_scalar_add(kT[:, eo, sc * E:(sc + 1) * E], pk,
                                            bkt[:, eo:eo + 1])

        v_sb = big.tile([P, 16, E], BF16, tag="v_sb")
        for so in range(16):
            pv = ps_sm.tile([P, E], F32, tag="ps", name="pv")
            for k in range(4):
                nc.tensor.matmul(pv, xbT[:, k, so * P:(so + 1) * P], wv_t[:, k, :],
                                 start=(k == 0), stop=(k == 3))
            nc.vector.tensor_tensor(v_sb[:, so, :], pv, bv_bc, ADD)

        def warm():
            # Full 128x128-array bf16 matmul; keeps the HAM clock-gate at
            # K=8/8 (half-array score/attn matmuls do not register as PE
            # activity, so without these the PE drops to 1.2 GHz).
            wp = ps_warm.tile([P, 256], F32, tag="warm", name="wp")
            nc.tensor.matmul(wp, ident, kT[:, 0, 0:256], start=True, stop=True)

        # concat (unnormalized attn^T, head-major, rows 0:64) reuses xbT's
        # pool slot -- xbT is dead once the projections are done.
        concat = big.tile([64, H, SQ], BF16, tag="xbT", name="concat")
        sums = stats.tile([P, 32], F32)
        inv = stats.tile([P, 32], F32, tag="inv")

        # ---- head loop -------------------------------------------------
        for h in range(H):
            hp = h % 2
            ko = h // 2
            lo, hi = 64 * hp, 64 * hp + 64
            at_ps = ps_sm.tile([64, SQ], F32, tag="ps", name="at_ps")
            for i in range(4):
                # orientation 1, query block i: 4 matmuls -> FD=2048 exp
                qb = i
                c = h * 4 + qb
                ps1 = ps_s1.tile([P, 4, E], F32, tag="s1", name="ps1")
                for sc in range(4):
                    nc.tensor.matmul(
                        ps1[:, sc, :],
                        qT[lo:hi, ko, qb * P:(qb + 1) * P],
                        kT[lo:hi, ko, sc * E:(sc + 1) * E],
                        start=True, stop=True,
                        tile_position=(64 * hp, 0),
                    )
                hslab = slabs.tile([P, S], F32, tag="hslab", name="hslab")
                warm()
                nc.scalar.activation(
                    hslab, ps1.rearrange("p a b -> p (a b)"), EXP,
                    scale=SCALE, accum_out=sums[:, c:c + 1],
                )
                nc.vector.reciprocal(inv[:, c:c + 1], sums[:, c:c + 1])
                nc.vector.tensor_scalar_mul(hslab, hslab, inv[:, c:c + 1])
                nc.sync.dma_start(w_out[h, qb * P:(qb + 1) * P, :], hslab)

                # orientation 2, two sk-block pairs + attn accumulation
                for sg in (2 * i, 2 * i + 1):
                    ps2 = ps_s2.tile([P, 2, E], F32, tag="s2", name="ps2")
                    for j in range(2):
                        so = sg * 2 + j
                        nc.tensor.matmul(
                            ps2[:, j, :],
                            kT[lo:hi, ko, so * P:(so + 1) * P],
                            qT[lo:hi, ko, :],
                            start=True, stop=True,
                            tile_position=(64 * hp, 0),
                        )
                    warm()
                    ptc = ptp.tile([P, 2, E], BF16, tag="pt", name="ptc")
                    nc.scalar.activation(ptc, ps2, EXP, scale=SCALE)
                    for j in range(2):
                        so = sg * 2 + j
                        nc.tensor.matmul(
                            at_ps,
                            v_sb[:, so, h * DH:(h + 1) * DH],
                            ptc[:, j, :],
                            start=(sg == 0 and j == 0), stop=(sg == 7 and j == 1),
                        )

            # replicate 1/sums across partitions: PE transpose -> DRAM
            # bounce -> broadcast read (DVE cannot broadcast across
            # partitions).
            ivp = ps_warm.tile([4, P], F32, tag="warm", name="ivp")
            nc.tensor.transpose(ivp, inv[:, h * 4:(h + 1) * 4], identf)
            ivs = invt.tile([4, P], F32, tag="ivs", name="ivs")
            nc.vector.tensor_copy(ivs, ivp)
            dinv = dramp.tile([4, P], F32, tag="dinv", name="dinv")
            nc.sync.dma_start(dinv, ivs)
            ibc = invbc.tile([64, SQ], F32, tag="ibc", name="ibc")
            nc.sync.dma_start(
                ibc, dinv.rearrange("a b -> (a b)")[None, :].to_broadcast((64, SQ)))

            nc.vector.tensor_tensor(concat[:, h, :], at_ps, ibc, MUL)

        # ---- output projection ----------------------------------------
        for qb in range(4):
            po = ps_sm.tile([P, E], F32, tag="ps", name="po")
            for h in range(H):
                nc.tensor.matmul(po, concat[:, h, qb * P:(qb + 1) * P], wo_t[:, h, :],
                                 start=(h == 0), stop=(h == H - 1))
            osb = osbp.tile([P, E], F32, tag="osb", name="osb")
            nc.vector.tensor_tensor(osb, po, bo_bc, ADD)
            nc.sync.dma_start(o_out[qb * P:(qb + 1) * P, :], osb)

    nc.compile()
    return nc


def _get_nc():
    if "nc" not in _CACHE:
        _CACHE["nc"] = _build()
    return _CACHE["nc"]


def _make_in_maps(x, Wq, bq, Wk, bk, Wv, bv, Wo, bo):
    f = lambda a: np.ascontiguousarray(np.asarray(a, dtype=np.float32))
    x = f(x)
    common = dict(wq=f(Wq), wk=f(Wk), wv=f(Wv), wo=f(Wo),
                  bq=f(bq), bk=f(bk), bv=f(bv), bo=f(bo))
    in_maps = []
    for c in range(NCORES):
        b, qc = c // 4, c % 4
        in_maps.append(dict(
            xb=np.ascontiguousarray(x[b]),
            xq=np.ascontiguousarray(x[b, qc * SQ:(qc + 1) * SQ]),
            **common))
    return in_maps


def _run(in_maps, **kwargs):
    from concourse.bass_utils import run_bass_kernel_spmd
    nc = _get_nc()
    return run_bass_kernel_spmd(nc, in_maps, core_ids=list(range(NCORES)), **kwargs)


def _assemble(results):
    out = np.empty((B, S, E), dtype=np.float32)
    weights = np.empty((B, H, S, S), dtype=np.float32)
    for c in range(NCORES):
        b, qc = c // 4, c % 4
        out[b, qc * SQ:(qc + 1) * SQ, :] = results[c]["o_out"]
        weights[b, :, qc * SQ:(qc + 1) * SQ, :] = results[c]["w_out"]
    return out, weights


def kernel(x, Wq, bq, Wk, bk, Wv, bv, Wo, bo):
    in_maps = _make_in_maps(x, Wq, bq, Wk, bk, Wv, bv, Wo, bo)
    res = _run(in_maps)
    return _assemble(res.results)


# revision 12
# speedup vs baseline: 1.3394x; 1.3394x over previous
"""MultiHeadAttention Trainium2 kernel (8 NeuronCores, SPMD, no collectives).

Problem: B=2, S=2048, E=512, H=8, Dh=64.  reference returns (out, weights):
  out     [B, S, E]      = softmax(q k^T / sqrt(Dh)) v  projected by Wo
  weights [B, H, S, S]   = the softmax attention weights (f32, 134 MB -> the
                           memory-traffic bottleneck; target_regime=memory)

Sharding: query-chunk data parallel.  Core c handles batch b=c//4 and query
rows [qc*512, (qc+1)*512) with qc=c%4, for ALL heads.  Each core computes
k/v for its full batch (recompute instead of collectives) and owns disjoint
slices of both outputs, so there is no cross-core communication.

Per-core plan (matmuls in bf16: full PE rate, FWL weight loads, keeps the
HAM clock-gate warm; accumulation is always f32 in PSUM):
  xbT = transpose(x[b]) via PE         [E,S] layout, E on partitions
  qT  = Wq^T xqT + bq                  [E,SQ]    (stored bf16)
  kT  = Wk^T xbT + bk                  [E,S]     (stored bf16)
  v   = x[b] Wv + bv                   [S,E]     (stored bf16)
  per head h:
    orientation 1 (weights output): scores[q,k] -> one FD=2048 exp on ACT
      (PSUM -> SBUF f32, accumulate register -> row sums) -> x(1/sum)
      (DVE per-partition scalar) -> DMA to w_out.  Max-subtraction is
      skipped (scores ~ N(0,1), exp range is tiny).
    1/sums are PE-transposed and DMA-bounced through DRAM into a
      partition-replicated tile (DVE cannot broadcast across partitions)
      to normalize the attention path.
    orientation 2 (attn): scoresT[k,q] -> exp -> pT (bf16); attn^T
      accumulated on PE with lhsT = v slice, dst partitions 0:64.
  out = sum_h concat_h^T Wo_h + bo -> DMA o_out   (K=64 chunks per head).
"""

import numpy as np

P = 128
S = 2048
E = 512
H = 8
DH = 64
SQ = 512          # queries per core
B = 2
NCORES = 8
SCALE = 0.125     # 1/sqrt(DH)

_CACHE = {}


def _build():
    import concourse.mybir as mybir
    import concourse.tile as tile
    from concourse import bacc
    from concourse.masks import make_identity
    from contextlib import ExitStack

    F32 = mybir.dt.float32
    BF16 = mybir.dt.bfloat16
    EXP = mybir.ActivationFunctionType.Exp
    MUL = mybir.AluOpType.mult
    ADD = mybir.AluOpType.add

    nc = bacc.Bacc(
        "TRN2", target_bir_lowering=False, debug=False,
        enable_asserts=False, num_devices=NCORES,
    )

    xb = nc.dram_tensor("xb", [S, E], F32, kind="ExternalInput").ap()
    xq = nc.dram_tensor("xq", [SQ, E], F32, kind="ExternalInput").ap()
    wq = nc.dram_tensor("wq", [E, E], F32, kind="ExternalInput").ap()
    wk = nc.dram_tensor("wk", [E, E], F32, kind="ExternalInput").ap()
    wv = nc.dram_tensor("wv", [E, E], F32, kind="ExternalInput").ap()
    wo = nc.dram_tensor("wo", [E, E], F32, kind="ExternalInput").ap()
    bq = nc.dram_tensor("bq", [E], F32, kind="ExternalInput").ap()
    bk = nc.dram_tensor("bk", [E], F32, kind="ExternalInput").ap()
    bv = nc.dram_tensor("bv", [E], F32, kind="ExternalInput").ap()
    bo = nc.dram_tensor("bo", [E], F32, kind="ExternalInput").ap()
    w_out = nc.dram_tensor("w_out", [H, SQ, S], F32, kind="ExternalOutput").ap()
    o_out = nc.dram_tensor("o_out", [SQ, E], F32, kind="ExternalOutput").ap()

    with tile.TileContext(nc) as tc, ExitStack() as ctx:
        consts = ctx.enter_context(tc.tile_pool(name="consts", bufs=1))
        ld = ctx.enter_context(tc.tile_pool(name="ld", bufs=4))
        wts = ctx.enter_context(tc.tile_pool(name="wts", bufs=1))
        big = ctx.enter_context(tc.tile_pool(name="big", bufs=1))
        slabs = ctx.enter_context(tc.tile_pool(name="slabs", bufs=5))
        ptp = ctx.enter_context(tc.tile_pool(name="ptp", bufs=4))
        stats = ctx.enter_context(tc.tile_pool(name="stats", bufs=1))
        invt = ctx.enter_context(tc.tile_pool(name="invt", bufs=2))
        invbc = ctx.enter_context(tc.tile_pool(name="invbc", bufs=3))
        osbp = ctx.enter_context(tc.tile_pool(name="osbp", bufs=2))
        dramp = ctx.enter_context(tc.tile_pool(name="dramp", bufs=2, space="DRAM"))
        # PSUM: ps_sm 2 banks (phase 0/1 copies, the two per-pair attn
        # accumulators, out projection) + s1 4 banks (pair-shared) + s2 2
        # banks (pair-shared; also hosts the tiny inv transposes) = 8.
        ps_sm = ctx.enter_context(tc.tile_pool(name="ps_sm", bufs=2, space="PSUM"))
        ps_s1 = ctx.enter_context(tc.tile_pool(name="ps_s1", bufs=1, space="PSUM"))
        ps_s2 = ctx.enter_context(tc.tile_pool(name="ps_s2", bufs=1, space="PSUM"))

        # ---- constants -------------------------------------------------
        ident = consts.tile([P, P], BF16)
        make_identity(nc, ident)
        identf = consts.tile([P, P], F32)
        make_identity(nc, identf)
        bqt = consts.tile([P, 4], F32)
        nc.sync.dma_start(bqt, bq.rearrange("(o i) -> i o", i=P))
        bkt = consts.tile([P, 4], F32)
        nc.sync.dma_start(bkt, bk.rearrange("(o i) -> i o", i=P))
        bv_bc = consts.tile([P, E], F32)
        nc.sync.dma_start(bv_bc, bv[None, :].to_broadcast((P, E)))
        bo_bc = consts.tile([P, E], F32)
        nc.sync.dma_start(bo_bc, bo[None, :].to_broadcast((P, E)))

        # ---- weights (cast to bf16 during DMA; SWDGE) ------------------
        wv_t = wts.tile([P, 4, E], BF16, tag="wv")
        nc.gpsimd.dma_start(wv_t, wv.rearrange("(ko ki) n -> ki ko n", ki=P))
        # Wo head-major at partitions 0:64 (out-projection contracts per-head
        # K=64 chunks; matmul dst must start at partition 0).
        wo_t = wts.tile([64, H, E], BF16, tag="wo")
        nc.gpsimd.dma_start(wo_t, wo.rearrange("(h ki) n -> ki h n", ki=64))

        # ---- transpose x (bf16 via DMA cast, PE transpose) -------------
        xbT = big.tile([P, 4, S], BF16, tag="xbT")
        xb_r = xb.rearrange("(sc j si) e -> si sc j e", si=P, j=2)
        for sc in range(8):
            ch = ld.tile([P, 2, E], BF16, tag="ldc", name="xbc")
            nc.gpsimd.dma_start(ch, xb_r[:, sc])
            for j in range(2):
                so = sc * 2 + j
                tp = ps_sm.tile([P, 4, P], BF16, tag="ps", name="tp")
                for eo in range(4):
                    nc.tensor.transpose(tp[:, eo, :], ch[:, j, eo * P:(eo + 1) * P], ident)
                nc.vector.tensor_copy(xbT[:, :, so * P:(so + 1) * P], tp)

        xqT = big.tile([P, 4, SQ], BF16, tag="xqT")
        xq_r = xq.rearrange("(sc j si) e -> si sc j e", si=P, j=2)
        for sc in range(2):
            ch = ld.tile([P, 2, E], BF16, tag="ldc", name="xqc")
            nc.gpsimd.dma_start(ch, xq_r[:, sc])
            for j in range(2):
                so = sc * 2 + j
                tp = ps_sm.tile([P, 4, P], BF16, tag="ps", name="tq")
                for eo in range(4):
                    nc.tensor.transpose(tp[:, eo, :], ch[:, j, eo * P:(eo + 1) * P], ident)
                nc.vector.tensor_copy(xqT[:, :, so * P:(so + 1) * P], tp)

        wq_t = ld.tile([P, 4, E], BF16, tag="ldc", name="wq_t")
        nc.gpsimd.dma_start(wq_t, wq.rearrange("(ko ki) n -> ki ko n", ki=P))
        wk_t = ld.tile([P, 4, E], BF16, tag="ldc", name="wk_t")
        nc.gpsimd.dma_start(wk_t, wk.rearrange("(ko ki) n -> ki ko n", ki=P))

        # ---- projections (bf16 matmuls, f32 PSUM accumulation) ---------
        qT = big.tile([P, 4, SQ], BF16, tag="qT")
        for eo in range(4):
            pq = ps_sm.tile([P, E], F32, tag="ps", name="pq")
            for k in range(4):
                nc.tensor.matmul(pq, wq_t[:, k, eo * P:(eo + 1) * P], xqT[:, k, :],
                                 start=(k == 0), stop=(k == 3))
            nc.vector.tensor_scalar_add(qT[:, eo, :], pq, bqt[:, eo:eo + 1])

        kT = big.tile([P, 4, S], BF16, tag="kT")
        for eo in range(4):
            for sc in range(4):
                pk = ps_sm.tile([P, E], F32, tag="ps", name="pk")
                for k in range(4):
                    nc.tensor.matmul(pk, wk_t[:, k, eo * P:(eo + 1) * P],
                                     xbT[:, k, sc * E:(sc + 1) * E],
                                     start=(k == 0), stop=(k == 3))
                nc.vector.tensor_scalar_add(kT[:, eo, sc * E:(sc + 1) * E], pk,
                                            bkt[:, eo:eo + 1])

        v_sb = big.tile([P, 16, E], BF16, tag="v_sb")
        for so in range(16):
            pv = ps_sm.tile([P, E], F32, tag="ps", name="pv")
            for k in range(4):
                nc.tensor.matmul(pv, xbT[:, k, so * P:(so + 1) * P], wv_t[:, k, :],
                                 start=(k == 0), stop=(k == 3))
            nc.vector.tensor_tensor(v_sb[:, so, :], pv, bv_bc, ADD)

        # concat (unnormalized attn^T, head-major, rows 0:64) reuses xbT's
        # pool slot -- xbT is dead once the projections are done.
        concat = big.tile([64, H, SQ], BF16, tag="xbT", name="concat")
        sums = stats.tile([P, 64], F32)
        sumf = stats.tile([P, 32], F32, tag="sumf")
        inv = stats.tile([P, 32], F32, tag="inv")

        # ---- head loop: heads processed in even/odd pairs so their
        # K=64 score matmuls run CONCURRENTLY on PE row groups 0:63 and
        # 64:127 (head 2t+hp lives at partition rows 64*hp of E-block t).
        for t in range(4):
            h0, h1 = 2 * t, 2 * t + 1
            at0 = ps_sm.tile([64, SQ], F32, tag="ps", name="at0")
            at1 = ps_sm.tile([64, SQ], F32, tag="ps", name="at1")
            ats = {h0: at0, h1: at1}
            for qb in range(4):
                hslabs = {h0: [], h1: []}
                for half in range(2):
                    # 4 banks: [128,4,512] = h0's two sc chunks in banks
                    # 0-1, h1's in banks 2-3; even/odd matmuls interleaved
                    # to overlap on disjoint PE row groups.
                    ps1 = ps_s1.tile([P, 4, E], F32, tag="s1", name="ps1")
                    for j in range(2):
                        sc = 2 * half + j
                        for hp in range(2):
                            nc.tensor.matmul(
                                ps1[:, 2 * hp + j, :],
                                qT[64 * hp:64 * hp + 64, t, qb * P:(qb + 1) * P],
                                kT[64 * hp:64 * hp + 64, t, sc * E:(sc + 1) * E],
                                start=True, stop=True,
                                tile_position=(64 * hp, 0),
                            )
                    for hp, h in ((0, h0), (1, h1)):
                        c2 = (h * 4 + qb) * 2 + half
                        hslab = slabs.tile([P, 1024], F32, tag="hslab", name="hslab")
                        nc.scalar.activation(
                            hslab,
                            ps1[:, 2 * hp:2 * hp + 2, :].rearrange("p a b -> p (a b)"),
                            EXP, scale=SCALE, accum_out=sums[:, c2:c2 + 1],
                        )
                        hslabs[h].append(hslab)
                for h in (h0, h1):
                    c = h * 4 + qb
                    nc.vector.tensor_tensor(
                        sumf[:, c:c + 1], sums[:, 2 * c:2 * c + 1],
                        sums[:, 2 * c + 1:2 * c + 2], ADD)
                    nc.vector.reciprocal(inv[:, c:c + 1], sumf[:, c:c + 1])
                    for half in range(2):
                        hs = hslabs[h][half]
                        nc.vector.tensor_scalar_mul(hs, hs, inv[:, c:c + 1])
                        nc.sync.dma_start(
                            w_out[h, qb * P:(qb + 1) * P,
                                  half * 1024:(half + 1) * 1024], hs)

                # orientation 2: 4 sk-blocks, both heads per chunk
                for so in range(4 * qb, 4 * qb + 4):
                    ps2 = ps_s2.tile([P, 2, E], F32, tag="s2", name="ps2")
                    for hp in range(2):
                        nc.tensor.matmul(
                            ps2[:, hp, :],
                            kT[64 * hp:64 * hp + 64, t, so * P:(so + 1) * P],
                            qT[64 * hp:64 * hp + 64, t, :],
                            start=True, stop=True,
                            tile_position=(64 * hp, 0),
                        )
                    ptc = ptp.tile([P, 2, E], BF16, tag="pt", name="ptc")
                    nc.scalar.activation(ptc, ps2, EXP, scale=SCALE)
                    for hp, h in ((0, h0), (1, h1)):
                        nc.tensor.matmul(
                            ats[h],
                            v_sb[:, so, h * DH:(h + 1) * DH],
                            ptc[:, hp, :],
                            start=(so == 0), stop=(so == 15),
                        )

            # replicate 1/sums across partitions: PE transpose -> DRAM
            # bounce -> broadcast read (DVE cannot broadcast across
            # partitions).
            for h in (h0, h1):
                ivp = ps_s2.tile([4, P], F32, tag="s2", name="ivp")
                nc.tensor.transpose(ivp, inv[:, h * 4:(h + 1) * 4], identf)
                ivs = invt.tile([4, P], F32, tag="ivs", name="ivs")
                nc.vector.tensor_copy(ivs, ivp)
                dinv = dramp.tile([4, P], F32, tag="dinv", name="dinv")
                nc.sync.dma_start(dinv, ivs)
                ibc = invbc.tile([64, SQ], F32, tag="ibc", name="ibc")
                nc.sync.dma_start(
                    ibc, dinv.rearrange("a b -> (a b)")[None, :].to_broadcast((64, SQ)))
                nc.vector.tensor_tensor(concat[:, h, :], ats[h], ibc, MUL)

        # ---- output projection ----------------------------------------
        for qb in range(4):
            po = ps_sm.tile([P, E], F32, tag="ps", name="po")
            for h in range(H):
                nc.tensor.matmul(po, concat[:, h, qb * P:(qb + 1) * P], wo_t[:, h, :],
                                 start=(h == 0), stop=(h == H - 1))
            osb = osbp.tile([P, E], F32, tag="osb", name="osb")
            nc.vector.tensor_tensor(osb, po, bo_bc, ADD)
            nc.sync.dma_start(o_out[qb * P:(qb + 1) * P, :], osb)

    nc.compile()
    return nc


def _get_nc():
    if "nc" not in _CACHE:
        _CACHE["nc"] = _build()
    return _CACHE["nc"]


def _make_in_maps(x, Wq, bq, Wk, bk, Wv, bv, Wo, bo):
    f = lambda a: np.ascontiguousarray(np.asarray(a, dtype=np.float32))
    x = f(x)
    common = dict(wq=f(Wq), wk=f(Wk), wv=f(Wv), wo=f(Wo),
                  bq=f(bq), bk=f(bk), bv=f(bv), bo=f(bo))
    in_maps = []
    for c in range(NCORES):
        b, qc = c // 4, c % 4
        in_maps.append(dict(
            xb=np.ascontiguousarray(x[b]),
            xq=np.ascontiguousarray(x[b, qc * SQ:(qc + 1) * SQ]),
            **common))
    return in_maps


def _run(in_maps, **kwargs):
    from concourse.bass_utils import run_bass_kernel_spmd
    nc = _get_nc()
    return run_bass_kernel_spmd(nc, in_maps, core_ids=list(range(NCORES)), **kwargs)


def _assemble(results):
    out = np.empty((B, S, E), dtype=np.float32)
    weights = np.empty((B, H, S, S), dtype=np.float32)
    for c in range(NCORES):
        b, qc = c // 4, c % 4
        out[b, qc * SQ:(qc + 1) * SQ, :] = results[c]["o_out"]
        weights[b, :, qc * SQ:(qc + 1) * SQ, :] = results[c]["w_out"]
    return out, weights


def kernel(x, Wq, bq, Wk, bk, Wv, bv, Wo, bo):
    in_maps = _make_in_maps(x, Wq, bq, Wk, bk, Wv, bv, Wo, bo)
    res = _run(in_maps)
    return _assemble(res.results)


# revision 15
# speedup vs baseline: 1.4990x; 1.1192x over previous
"""MultiHeadAttention Trainium2 kernel (8 NeuronCores, SPMD, no collectives).

Problem: B=2, S=2048, E=512, H=8, Dh=64.  reference returns (out, weights):
  out     [B, S, E]      = softmax(q k^T / sqrt(Dh)) v  projected by Wo
  weights [B, H, S, S]   = the softmax attention weights (f32, 134 MB -> the
                           memory-traffic bottleneck; target_regime=memory)

Sharding: query-chunk data parallel.  Core c handles batch b=c//4 and query
rows [qc*512, (qc+1)*512) with qc=c%4, for ALL heads.  Each core computes
k/v for its full batch (recompute instead of collectives) and owns disjoint
slices of both outputs, so there is no cross-core communication.

Per-core plan (matmuls in bf16: full PE rate, FWL weight loads, keeps the
HAM clock-gate warm; accumulation is always f32 in PSUM):
  xbT = transpose(x[b]) via PE         [E,S] layout, E on partitions
  qT  = Wq^T xqT + bq                  [E,SQ]    (stored bf16)
  kT  = Wk^T xbT + bk                  [E,S]     (stored bf16)
  v   = x[b] Wv + bv                   [S,E]     (stored bf16)
  per head h:
    orientation 1 (weights output): scores[q,k] -> one FD=2048 exp on ACT
      (PSUM -> SBUF f32, accumulate register -> row sums) -> x(1/sum)
      (DVE per-partition scalar) -> DMA to w_out.  Max-subtraction is
      skipped (scores ~ N(0,1), exp range is tiny).
    1/sums are PE-transposed and DMA-bounced through DRAM into a
      partition-replicated tile (DVE cannot broadcast across partitions)
      to normalize the attention path.
    orientation 2 (attn): scoresT[k,q] -> exp -> pT (bf16); attn^T
      accumulated on PE with lhsT = v slice, dst partitions 0:64.
  out = sum_h concat_h^T Wo_h + bo -> DMA o_out   (K=64 chunks per head).
"""

import numpy as np

P = 128
S = 2048
E = 512
H = 8
DH = 64
SQ = 512          # queries per core
B = 2
NCORES = 8
SCALE = 0.125     # 1/sqrt(DH)

_CACHE = {}


def _build():
    import concourse.mybir as mybir
    import concourse.tile as tile
    from concourse import bacc
    from concourse.masks import make_identity
    from contextlib import ExitStack

    F32 = mybir.dt.float32
    BF16 = mybir.dt.bfloat16
    EXP = mybir.ActivationFunctionType.Exp
    MUL = mybir.AluOpType.mult
    ADD = mybir.AluOpType.add

    nc = bacc.Bacc(
        "TRN2", target_bir_lowering=False, debug=False,
        enable_asserts=False, num_devices=NCORES,
    )

    xb = nc.dram_tensor("xb", [S, E], F32, kind="ExternalInput").ap()
    xq = nc.dram_tensor("xq", [SQ, E], F32, kind="ExternalInput").ap()
    wq = nc.dram_tensor("wq", [E, E], F32, kind="ExternalInput").ap()
    wk = nc.dram_tensor("wk", [E, E], F32, kind="ExternalInput").ap()
    wv = nc.dram_tensor("wv", [E, E], F32, kind="ExternalInput").ap()
    wo = nc.dram_tensor("wo", [E, E], F32, kind="ExternalInput").ap()
    bq = nc.dram_tensor("bq", [E], F32, kind="ExternalInput").ap()
    bk = nc.dram_tensor("bk", [E], F32, kind="ExternalInput").ap()
    bv = nc.dram_tensor("bv", [E], F32, kind="ExternalInput").ap()
    bo = nc.dram_tensor("bo", [E], F32, kind="ExternalInput").ap()
    w_out = nc.dram_tensor("w_out", [H, SQ, S], F32, kind="ExternalOutput").ap()
    o_out = nc.dram_tensor("o_out", [SQ, E], F32, kind="ExternalOutput").ap()

    with tile.TileContext(nc) as tc, ExitStack() as ctx:
        consts = ctx.enter_context(tc.tile_pool(name="consts", bufs=1))
        ld = ctx.enter_context(tc.tile_pool(name="ld", bufs=4))
        wts = ctx.enter_context(tc.tile_pool(name="wts", bufs=1))
        big = ctx.enter_context(tc.tile_pool(name="big", bufs=1))
        slabs = ctx.enter_context(tc.tile_pool(name="slabs", bufs=4))
        ptp = ctx.enter_context(tc.tile_pool(name="ptp", bufs=4))
        stats = ctx.enter_context(tc.tile_pool(name="stats", bufs=1))
        invt = ctx.enter_context(tc.tile_pool(name="invt", bufs=2))
        invbc = ctx.enter_context(tc.tile_pool(name="invbc", bufs=3))
        osbp = ctx.enter_context(tc.tile_pool(name="osbp", bufs=2))
        dramp = ctx.enter_context(tc.tile_pool(name="dramp", bufs=2, space="DRAM"))
        # PSUM: ps_sm 2 banks (phase 0/1 copies, the two per-pair attn
        # accumulators, out projection) + s1 4 banks (pair-shared) + s2 2
        # banks (pair-shared; also hosts the tiny inv transposes) = 8.
        ps_sm = ctx.enter_context(tc.tile_pool(name="ps_sm", bufs=2, space="PSUM"))
        ps_s1 = ctx.enter_context(tc.tile_pool(name="ps_s1", bufs=1, space="PSUM"))
        ps_s2 = ctx.enter_context(tc.tile_pool(name="ps_s2", bufs=1, space="PSUM"))

        # ---- constants -------------------------------------------------
        ident = consts.tile([P, P], BF16)
        make_identity(nc, ident)
        identf = consts.tile([P, P], F32)
        make_identity(nc, identf)
        bqt = consts.tile([P, 4], F32)
        nc.sync.dma_start(bqt, bq.rearrange("(o i) -> i o", i=P))
        bkt = consts.tile([P, 4], F32)
        nc.sync.dma_start(bkt, bk.rearrange("(o i) -> i o", i=P))
        bv_bc = consts.tile([P, E], F32)
        nc.sync.dma_start(bv_bc, bv[None, :].to_broadcast((P, E)))
        bo_bc = consts.tile([P, E], F32)
        nc.sync.dma_start(bo_bc, bo[None, :].to_broadcast((P, E)))

        # ---- weights (cast to bf16 during DMA; SWDGE) ------------------
        wv_t = wts.tile([P, 4, E], BF16, tag="wv")
        nc.gpsimd.dma_start(wv_t, wv.rearrange("(ko ki) n -> ki ko n", ki=P))
        wo_t = wts.tile([P, 4, E], BF16, tag="wo")
        nc.gpsimd.dma_start(wo_t, wo.rearrange("(ko ki) n -> ki ko n", ki=P))

        # ---- transpose x (bf16 via DMA cast, PE transpose) -------------
        xbT = big.tile([P, 4, S], BF16, tag="xbT")
        xb_r = xb.rearrange("(sc j si) e -> si sc j e", si=P, j=2)
        for sc in range(8):
            ch = ld.tile([P, 2, E], BF16, tag="ldc", name="xbc")
            nc.gpsimd.dma_start(ch, xb_r[:, sc])
            for j in range(2):
                so = sc * 2 + j
                tp = ps_sm.tile([P, 4, P], BF16, tag="ps", name="tp")
                for eo in range(4):
                    nc.tensor.transpose(tp[:, eo, :], ch[:, j, eo * P:(eo + 1) * P], ident)
                nc.vector.tensor_copy(xbT[:, :, so * P:(so + 1) * P], tp)

        xqT = big.tile([P, 4, SQ], BF16, tag="xqT")
        xq_r = xq.rearrange("(sc j si) e -> si sc j e", si=P, j=2)
        for sc in range(2):
            ch = ld.tile([P, 2, E], BF16, tag="ldc", name="xqc")
            nc.gpsimd.dma_start(ch, xq_r[:, sc])
            for j in range(2):
                so = sc * 2 + j
                tp = ps_sm.tile([P, 4, P], BF16, tag="ps", name="tq")
                for eo in range(4):
                    nc.tensor.transpose(tp[:, eo, :], ch[:, j, eo * P:(eo + 1) * P], ident)
                nc.vector.tensor_copy(xqT[:, :, so * P:(so + 1) * P], tp)

        wq_t = ld.tile([P, 4, E], BF16, tag="ldc", name="wq_t")
        nc.gpsimd.dma_start(wq_t, wq.rearrange("(ko ki) n -> ki ko n", ki=P))
        wk_t = ld.tile([P, 4, E], BF16, tag="ldc", name="wk_t")
        nc.gpsimd.dma_start(wk_t, wk.rearrange("(ko ki) n -> ki ko n", ki=P))

        # ---- projections (bf16 matmuls, f32 PSUM accumulation) ---------
        qT = big.tile([P, 4, SQ], BF16, tag="qT")
        for eo in range(4):
            pq = ps_sm.tile([P, E], F32, tag="ps", name="pq")
            for k in range(4):
                nc.tensor.matmul(pq, wq_t[:, k, eo * P:(eo + 1) * P], xqT[:, k, :],
                                 start=(k == 0), stop=(k == 3))
            nc.vector.tensor_scalar_add(qT[:, eo, :], pq, bqt[:, eo:eo + 1])

        kT = big.tile([P, 4, S], BF16, tag="kT")
        for eo in range(4):
            for sc in range(4):
                pk = ps_sm.tile([P, E], F32, tag="ps", name="pk")
                for k in range(4):
                    nc.tensor.matmul(pk, wk_t[:, k, eo * P:(eo + 1) * P],
                                     xbT[:, k, sc * E:(sc + 1) * E],
                                     start=(k == 0), stop=(k == 3))
                nc.vector.tensor_scalar_add(kT[:, eo, sc * E:(sc + 1) * E], pk,
                                            bkt[:, eo:eo + 1])

        v_sb = big.tile([P, 16, E], BF16, tag="v_sb")
        for so in range(16):
            pv = ps_sm.tile([P, E], F32, tag="ps", name="pv")
            for k in range(4):
                nc.tensor.matmul(pv, xbT[:, k, so * P:(so + 1) * P], wv_t[:, k, :],
                                 start=(k == 0), stop=(k == 3))
            nc.vector.tensor_tensor(v_sb[:, so, :], pv, bv_bc, ADD)

        # concatT (unnormalized attn^T, E-major: head h at partitions
        # 64*(h%2) of eo-block h//2) reuses xbT's pool slot -- xbT is dead
        # once the projections are done.
        concatT = big.tile([P, 4, SQ], BF16, tag="xbT", name="concatT")
        sums = stats.tile([P, 64], F32)
        sumf = stats.tile([P, 32], F32, tag="sumf")
        inv = stats.tile([P, 32], F32, tag="inv")

        # ---- head loop: heads processed in even/odd pairs so their
        # K=64 score matmuls run CONCURRENTLY on PE row groups 0:63 and
        # 64:127 (head 2t+hp lives at partition rows 64*hp of E-block t);
        # attention matmuls likewise pair on column groups 0:63 / 64:127
        # and accumulate into one shared PSUM bank via per-element
        # has_written (single start=True clears the bank once).
        for t in range(4):
            h0, h1 = 2 * t, 2 * t + 1
            at_pair = ps_sm.tile([P, SQ], F32, tag="ps", name="at_pair")
            # Bank-clearing dummy: one start=True matmul writing column 0
            # across all 128 partitions.  Its write overlaps both heads'
            # first real matmul (WAW -> ordered first), clears has_written
            # for the whole bank, and every real matmul then uses
            # start=False with per-element overwrite-then-accumulate.
            nc.tensor.matmul(at_pair[:, 0:1], ident, kT[:, 0, 0:1],
                             start=True, stop=True, skip_group_check=True)
            for qb in range(4):
                wslabs = {}
                for hp, h in ((0, h0), (1, h1)):
                    wslabs[h] = slabs.tile([P, S], F32, tag="wslab", name="wslab")
                for half in range(2):
                    # 4 banks: [128,4,512] = h0's two sc chunks in banks
                    # 0-1, h1's in banks 2-3; even/odd matmuls interleaved
                    # to overlap on disjoint PE row groups.
                    ps1 = ps_s1.tile([P, 4, E], F32, tag="s1", name="ps1")
                    for j in range(2):
                        sc = 2 * half + j
                        for hp in range(2):
                            nc.tensor.matmul(
                                ps1[:, 2 * hp + j, :],
                                qT[64 * hp:64 * hp + 64, t, qb * P:(qb + 1) * P],
                                kT[64 * hp:64 * hp + 64, t, sc * E:(sc + 1) * E],
                                start=True, stop=True,
                                tile_position=(64 * hp, 0),
                            )
                    for hp, h in ((0, h0), (1, h1)):
                        c2 = (h * 4 + qb) * 2 + half
                        nc.scalar.activation(
                            wslabs[h][:, half * 1024:(half + 1) * 1024],
                            ps1[:, 2 * hp:2 * hp + 2, :].rearrange("p a b -> p (a b)"),
                            EXP, scale=SCALE, accum_out=sums[:, c2:c2 + 1],
                        )
                for h in (h0, h1):
                    c = h * 4 + qb
                    nc.vector.tensor_tensor(
                        sumf[:, c:c + 1], sums[:, 2 * c:2 * c + 1],
                        sums[:, 2 * c + 1:2 * c + 2], ADD)
                    nc.vector.reciprocal(inv[:, c:c + 1], sumf[:, c:c + 1])
                    nc.vector.tensor_scalar_mul(wslabs[h], wslabs[h], inv[:, c:c + 1])
                    nc.sync.dma_start(w_out[h, qb * P:(qb + 1) * P, :], wslabs[h])

                # orientation 2: 4 sk-blocks, both heads per chunk; attn
                # pairs on column groups into the shared accumulator bank.
                for so in range(4 * qb, 4 * qb + 4):
                    ps2 = ps_s2.tile([P, 2, E], F32, tag="s2", name="ps2")
                    for hp in range(2):
                        nc.tensor.matmul(
                            ps2[:, hp, :],
                            kT[64 * hp:64 * hp + 64, t, so * P:(so + 1) * P],
                            qT[64 * hp:64 * hp + 64, t, :],
                            start=True, stop=True,
                            tile_position=(64 * hp, 0),
                        )
                    ptc = ptp.tile([P, 2, E], BF16, tag="pt", name="ptc")
                    nc.scalar.activation(ptc, ps2, EXP, scale=SCALE)
                    for hp, h in ((0, h0), (1, h1)):
                        nc.tensor.matmul(
                            at_pair[64 * hp:64 * hp + 64, :],
                            v_sb[:, so, h * DH:(h + 1) * DH],
                            ptc[:, hp, :],
                            start=False, stop=(so == 15 and hp == 1),
                            tile_position=(0, 64 * hp),
                            skip_group_check=True,
                        )

            # replicate 1/sums across partitions: PE transpose -> DRAM
            # bounce -> broadcast read (DVE cannot broadcast across
            # partitions).
            for hp, h in ((0, h0), (1, h1)):
                ivp = ps_s2.tile([4, P], F32, tag="s2", name="ivp")
                nc.tensor.transpose(ivp, inv[:, h * 4:(h + 1) * 4], identf)
                ivs = invt.tile([4, P], F32, tag="ivs", name="ivs")
                nc.vector.tensor_copy(ivs, ivp)
                dinv = dramp.tile([4, P], F32, tag="dinv", name="dinv")
                nc.sync.dma_start(dinv, ivs)
                ibc = invbc.tile([P, SQ], F32, tag="ibc", name="ibc")
                nc.sync.dma_start(
                    ibc, dinv.rearrange("a b -> (a b)")[None, :].to_broadcast((P, SQ)))
                nc.vector.tensor_tensor(
                    concatT[64 * hp:64 * hp + 64, t, :],
                    at_pair[64 * hp:64 * hp + 64, :],
                    ibc[64 * hp:64 * hp + 64, :], MUL)

        # ---- output projection ----------------------------------------
        for qb in range(4):
            po = ps_sm.tile([P, E], F32, tag="ps", name="po")
            for eo in range(4):
                nc.tensor.matmul(po, concatT[:, eo, qb * P:(qb + 1) * P], wo_t[:, eo, :],
                                 start=(eo == 0), stop=(eo == 3))
            osb = osbp.tile([P, E], F32, tag="osb", name="osb")
            nc.vector.tensor_tensor(osb, po, bo_bc, ADD)
            nc.sync.dma_start(o_out[qb * P:(qb + 1) * P, :], osb)

    nc.compile()
    return nc


def _get_nc():
    if "nc" not in _CACHE:
        _CACHE["nc"] = _build()
    return _CACHE["nc"]


def _make_in_maps(x, Wq, bq, Wk, bk, Wv, bv, Wo, bo):
    f = lambda a: np.ascontiguousarray(np.asarray(a, dtype=np.float32))
    x = f(x)
    common = dict(wq=f(Wq), wk=f(Wk), wv=f(Wv), wo=f(Wo),
                  bq=f(bq), bk=f(bk), bv=f(bv), bo=f(bo))
    in_maps = []
    for c in range(NCORES):
        b, qc = c // 4, c % 4
        in_maps.append(dict(
            xb=np.ascontiguousarray(x[b]),
            xq=np.ascontiguousarray(x[b, qc * SQ:(qc + 1) * SQ]),
            **common))
    return in_maps


def _run(in_maps, **kwargs):
    from concourse.bass_utils import run_bass_kernel_spmd
    nc = _get_nc()
    return run_bass_kernel_spmd(nc, in_maps, core_ids=list(range(NCORES)), **kwargs)


def _assemble(results):
    out = np.empty((B, S, E), dtype=np.float32)
    weights = np.empty((B, H, S, S), dtype=np.float32)
    for c in range(NCORES):
        b, qc = c // 4, c % 4
        out[b, qc * SQ:(qc + 1) * SQ, :] = results[c]["o_out"]
        weights[b, :, qc * SQ:(qc + 1) * SQ, :] = results[c]["w_out"]
    return out, weights


def kernel(x, Wq, bq, Wk, bk, Wv, bv, Wo, bo):
    in_maps = _make_in_maps(x, Wq, bq, Wk, bk, Wv, bv, Wo, bo)
    res = _run(in_maps)
    return _assemble(res.results)
